# revision 72
# baseline (speedup 1.0000x reference)
"""Multi-head attention (B=2, S=2048, D=1024, H=16) on 8 Trainium2 NeuronCores.

Sharding: core c handles batch b = c//4 and head-group g = c%4 (4 heads,
a 256-wide column slice of wq/wk/wv and row slice of wo).  Each core
computes a full [S, D] bf16 partial of the output projection; the host
sums the 4 partials per batch in f32 and adds the output bias.

Per-core kernel (all layouts chosen so softmax runs over the PSUM
partition axis and every DVE/ACT op stays partition-aligned):
  - QT/KT = (x @ w)^T in [head_dim, S] layout, bf16 matmuls; bias adds
    on the scalar engine (idle during the projection phase).
  - VH = x @ wv in natural [S, head_cols] layout, bf16, with a ones
    column appended per head (yields softmax denominators for free).
  - scores^T per head-pair via row-tiled (tile_position) K=64 matmuls,
    exp on the scalar engine (scale=1/8 folded in), probs in bf16.
  - ctx^T = VH_aug^T @ probs^T accumulated over S chunks, emitted two
    iterations behind the exp stream so the PE never waits on the
    in-flight exp; row 64 of the PSUM tile is the softmax denominator.
  - normalize: denominator row -> block-transpose -> 32-lane strided
    reciprocal -> transpose back -> bf16 K=1 replicate matmul -> DVE
    multiply (a row-shaped DVE reciprocal would cost ~6.5us since DVE
    op cost scales with free-dim length, not active lanes).
  - out partial = ctx^T.T @ wo_slice, bf16 out.

Schedule: the attention spine is ACT(exp)-bound at ~1.09us per key
chunk, so everything else rides its slack: the DMA rings are loaded in
exact consumption order (k first - scores need every key - then q0, v0,
q1-3, v1, wo); only K-proj and the first q chunk run before the spine;
the other q chunks, the entire V projection, and the output-projection
groups of finished query chunks are interleaved as fillers inside the
attention steps.  Softmax normalization lags one step behind attention,
off the PE critical path.
"""

import os
import sys

import ml_dtypes
import numpy as np

if "/opt/trn_rl_repo" not in sys.path:
    sys.path.insert(0, "/opt/trn_rl_repo")

B, S, D, H = 2, 2048, 1024, 16
DH = D // H  # 64
NCORES = 8
GC = 256  # column slice per core (4 heads)
NP = 2  # head pairs per core
KC = D // 128  # 8 contraction chunks
SQC = S // 512  # 4 query chunks
SKC = S // 128  # 16 key chunks

_CACHE = {}


def _build_program():
    import concourse.bass as bass
    import concourse.tile as tile
    from concourse import bacc, mybir

    F32 = mybir.dt.float32
    F32R = mybir.dt.float32r
    BF16 = mybir.dt.bfloat16
    F8 = mybir.dt.float8e4
    DR = mybir.MatmulPerfMode.DoubleRow
    EXP = mybir.ActivationFunctionType.Exp
    PSUM = bass.MemorySpace.PSUM

    nc = bacc.Bacc()

    qT = nc.dram_tensor("qT", (D, S), BF16, kind="ExternalInput").ap()
    kT = nc.dram_tensor("kT", (D, S), BF16, kind="ExternalInput").ap()
    vT = nc.dram_tensor("vT", (D, S), BF16, kind="ExternalInput").ap()
    wqs = nc.dram_tensor("wqs", (D, GC), BF16, kind="ExternalInput").ap()
    wks = nc.dram_tensor("wks", (D, GC), BF16, kind="ExternalInput").ap()
    wvs = nc.dram_tensor("wvs", (D, GC), BF16, kind="ExternalInput").ap()
    wos = nc.dram_tensor("wos", (GC, D), BF16, kind="ExternalInput").ap()
    bqs = nc.dram_tensor("bqs", (NP, 128, 1), F32, kind="ExternalInput").ap()
    bks = nc.dram_tensor("bks", (NP, 128, 1), F32, kind="ExternalInput").ap()
    bvs = nc.dram_tensor("bvs", (1, GC), F32R, kind="ExternalInput").ap()
    outp = nc.dram_tensor("outp", (S, D), BF16, kind="ExternalOutput").ap()

    with tile.TileContext(nc) as tc:
        with (
            tc.tile_pool(name="const", bufs=1) as const,
            tc.tile_pool(name="raw", bufs=1) as rawp,
            tc.tile_pool(name="probs", bufs=7) as probs,
            tc.tile_pool(name="small", bufs=2) as small,
            tc.tile_pool(name="outsb", bufs=3) as outsb,
            tc.tile_pool(name="pproj", bufs=2, space=PSUM) as pproj,
            tc.tile_pool(name="psc", bufs=2, space=PSUM) as psc,
            tc.tile_pool(name="pctx", bufs=1, space=PSUM) as pctx,
        ):
            # ---- constants / weights ----
            # wq/wk/wv as one [128, KC, GC] bf16 tile each (single DMA)
            wq_t = const.tile([128, KC, GC], BF16, name="wqt", tag="wqt")
            wk_t = const.tile([128, KC, GC], BF16, name="wkt", tag="wkt")
            wv_t = const.tile([128, KC, GC], BF16, name="wvt", tag="wvt")
            wo_t = [const.tile([128, D], BF16, name=f"wo{m}", tag=f"wo{m}") for m in range(NP)]
            bq_t = [const.tile([128, 1], F32, name=f"bq{m}", tag=f"bq{m}") for m in range(NP)]
            bk_t = [const.tile([128, 1], F32, name=f"bk{m}", tag=f"bk{m}") for m in range(NP)]
            bv_row = const.tile([1, GC], F32R, name="bvrow", tag="bvrow")
            ones_t = const.tile([128, 128], F32R, name="ones", tag="ones")

            # PE warmup chain: keeps the HAM clock-gate at 8/8 through the
            # initial input DMA window so projections start at 2.4 GHz.
            wu = const.tile([128, 512], BF16, name="wu", tag="wu")
            nc.vector.memset(wu[:], 0.0)
            wup = psc.tile([128, 1024], F32, name="sc", tag="sc")
            for w in range(16):
                nc.tensor.matmul(
                    wup[:, 0:512], wu[:, 0:128], wu[:],
                    start=(w == 0), stop=(w == 15),
                )

            # DMA rings are FIFO: enqueue in exact consumption order.  The
            # attention spine is gated by ALL of k (scores need every key)
            # plus just the first q chunk, so k streams first; q1-3 / v1 / wo
            # are consumed later as fillers inside the attention steps.
            nc.sync.dma_start(wk_t[:], wks.rearrange("(c p) g -> p c g", p=128))
            for m in range(NP):
                nc.sync.dma_start(bk_t[m][:], bks[m])
            # bv_row is 1KB and gates the bvb broadcast matmul right before
            # the attention spine -- it must not queue behind megabytes of k/q
            nc.sync.dma_start(bv_row[:], bvs[:])
            k_t = [
                rawp.tile([128, KC, 512], BF16, name=f"k{j}", tag=f"k{j}")
                for j in range(SQC)
            ]
            kr = kT.rearrange("(c p) s -> p c s", p=128)
            for j in range(SQC):
                nc.sync.dma_start(k_t[j][:], kr[:, :, j * 512 : (j + 1) * 512])
            nc.sync.dma_start(wq_t[:], wqs.rearrange("(c p) g -> p c g", p=128))
            for m in range(NP):
                nc.sync.dma_start(bq_t[m][:], bqs[m])
            q_t = [
                rawp.tile([128, KC, 512], BF16, name=f"q{j}", tag=f"q{j}")
                for j in range(SQC)
            ]
            qr = qT.rearrange("(c p) s -> p c s", p=128)
            nc.sync.dma_start(q_t[0][:], qr[:, :, 0:512])
            nc.sync.dma_start(wv_t[:], wvs.rearrange("(c p) g -> p c g", p=128))
            v_t = [
                rawp.tile([128, KC, 1024], BF16, name=f"v{j}", tag="vv", bufs=2)
                for j in range(2)
            ]
            vr = vT.rearrange("(c p) s -> p c s", p=128)
            nc.sync.dma_start(v_t[0][:], vr[:, :, 0:1024])
            nc.sync.dma_start(v_t[1][:], vr[:, :, 1024:2048])
            for j in range(1, SQC):
                nc.sync.dma_start(q_t[j][:], qr[:, :, j * 512 : (j + 1) * 512])
            for m in range(NP):
                nc.sync.dma_start(wo_t[m][:], wos[m * 128 : (m + 1) * 128, :])
            ones_f = const.tile([128, 128], F32, name="onesf", tag="onesf")
            nc.vector.memset(ones_f[:], 1.0)
            nc.vector.tensor_copy(ones_t[:], ones_f[:])
            ones_bf = const.tile([128, 128], BF16, name="onesbf", tag="onesbf")
            nc.vector.memset(ones_bf[:], 1.0)
            # staging tiles for the transposed-reciprocal chain: the softmax
            # denominators arrive as a [1, 1024] row (partition 64 of the ctx
            # PSUM); a row-shaped DVE reciprocal costs ~6.5us (cost scales
            # with free-dim, not lanes), so we block-transpose the row across
            # 32 partitions, reciprocal 32 lanes x 32 elems, and transpose
            # back. dn32/rT are memset once so the junk lanes the transposes
            # touch stay initialized.
            dn32 = const.tile([32, 1024], F32, name="dn32", tag="dn32")
            dnT = const.tile([32, 1024], F32, name="dnT", tag="dnT")
            rT = const.tile([32, 1024], F32, name="rT", tag="rT")
            rrow = const.tile([32, 1024], F32, name="rrow", tag="rrow")
            nc.vector.memset(dn32[:], 1.0)
            nc.vector.memset(rT[:], 1.0)
            bvb = const.tile([128, GC], F32, name="bvb", tag="bvb")

            # ---- persistent activation tiles ----
            QT = [const.tile([128, S], BF16, name=f"QT{m}", tag=f"QT{m}") for m in range(NP)]
            KT = [const.tile([128, S], BF16, name=f"KT{m}", tag=f"KT{m}") for m in range(NP)]
            # VH: [S-chunk][128, 4 heads, 66] bf16; col 64 = ones, col 65 pad
            VH = [const.tile([128, 4, 66], BF16, name=f"VH{i}", tag=f"VH{i}") for i in range(SKC)]
            ctxT = [const.tile([128, S], BF16, name=f"ctxT{m}", tag=f"ctxT{m}") for m in range(NP)]

            for i in range(SKC):
                nc.vector.memset(VH[i][:, :, 64:65], 1.0)

            # ---- phase 1: projections ----
            # Bias-adds for Q/K run on the scalar engine (idle during proj)
            # so the DVE stays clear for the attention-phase chains.
            def proj_pair(raw, col0, b_t, w_t, dst, nq, m):
                ps = pproj.tile([128, 512], F32, name="pj", tag="pj")
                for k in range(KC):
                    nc.tensor.matmul(
                        ps[:],
                        w_t[:, k, m * 128 : (m + 1) * 128],
                        raw[:, k, col0 : col0 + 512],
                        start=(k == 0),
                        stop=(k == KC - 1),
                    )
                nc.scalar.activation(
                    dst[m][:, nq * 512 : (nq + 1) * 512],
                    ps[:],
                    mybir.ActivationFunctionType.Identity,
                    bias=b_t[m][:],
                )

            # K proj first (m-outer: KT[0] completes at the halfway mark),
            # then only the q chunk the first attention step reads; the other
            # three q chunks are projected as fillers inside the early
            # attention steps, overlapping their own DMA.
            for m in range(NP):
                for nq in range(SQC):
                    proj_pair(k_t[nq], 0, bk_t, wk_t, KT, nq, m)
            for m in range(NP):
                proj_pair(q_t[0], 0, bq_t, wq_t, QT, 0, m)

            def q_filler(nq, m):
                def emit():
                    proj_pair(q_t[nq], 0, bq_t, wq_t, QT, nq, m)

                return emit
            # bv broadcast to all partitions: [128, GC] = ones[1,128].T @ bv[1,GC]
            # (emitted here, not earlier: the PE's in-order queue must not
            # wait on the bv_row DMA before the Q/K projections)
            bvp = pproj.tile([128, 512], F32, name="pj", tag="pj")
            nc.tensor.matmul(
                bvp[:, :GC], ones_t[0:1, 0:128], bv_row[:],
                start=True, stop=True,
            )
            nc.vector.tensor_copy(bvb[:], bvp[:, :GC])

            def v_chunk(i):
                # one 128-key chunk of the V projection ([S, GC] layout);
                # interleaved into the first attention step's iterations
                def emit():
                    v_raw = v_t[i // 8]
                    vcol = (i % 8) * 128
                    ps = pproj.tile([128, 512], F32, name="pj", tag="pj")
                    for k in range(KC):
                        nc.tensor.matmul(
                            ps[:, :GC],
                            v_raw[:, k, vcol : vcol + 128],
                            wv_t[:, k, :],
                            start=(k == 0),
                            stop=(k == KC - 1),
                        )
                    nc.vector.tensor_add(
                        VH[i][:, :, 0:64],
                        ps[:, :GC].rearrange("p (h d) -> p h d", h=4),
                        bvb[:].rearrange("p (h d) -> p h d", h=4),
                    )

                return emit

            # ---- phase 2 + 3: software-pipelined ----
            # attn_step(sq, m) ends with a fast PSUM evacuation; the slow
            # normalize chain (1-lane reciprocal on DVE) and the dependent
            # replicate/outproj matmuls are deferred one step so the PE's
            # in-order stream never blocks on the DVE chain.
            def attn_step(
                sq, m, fillers=None, pre_fillers=None, filler_start=3,
                filler_stride=2,
            ):
                fillers = fillers if fillers is not None else []
                pre_fillers = pre_fillers if pre_fillers is not None else []
                ctA = pctx.tile([128, 512], F32, name="ctA", tag="ctA")
                ctB = pctx.tile([128, 512], F32, name="ctB", tag="ctB")

                def ctx_mm(i, pb):
                    nc.tensor.matmul(
                        ctA[0:65, :], VH[i][:, 2 * m, 0:65], pb[:, 0:512],
                        start=(i == 0), stop=(i == SKC - 1),
                    )
                    nc.tensor.matmul(
                        ctB[0:65, :], VH[i][:, 2 * m + 1, 0:65], pb[:, 512:1024],
                        start=(i == 0), stop=(i == SKC - 1),
                    )

                # ctx matmuls run two iterations behind the scores/exp: the
                # PE consumes probs(i-2) while the scalar engine computes
                # exp(i), so the PE queue never blocks on the in-flight exp,
                # and interleaved V-projection chunks get a 2-iteration
                # cushion over their DMA.
                pending_ctx = []
                for i in range(SKC):
                    if pre_fillers:
                        pre_fillers.pop(0)()
                    if (
                        i % filler_stride == filler_stride - 1
                        and i >= filler_start
                        and fillers
                    ):
                        fillers.pop(0)()
                    sc = psc.tile([128, 1024], F32, name="sc", tag="sc")
                    nc.tensor.matmul(
                        sc[:, 0:512],
                        KT[m][0:64, i * 128 : (i + 1) * 128],
                        QT[m][0:64, sq * 512 : (sq + 1) * 512],
                        start=True, stop=True,
                    )
                    nc.tensor.matmul(
                        sc[:, 512:1024],
                        KT[m][64:128, i * 128 : (i + 1) * 128],
                        QT[m][64:128, sq * 512 : (sq + 1) * 512],
                        start=True, stop=True,
                        tile_position=(64, 0),
                    )
                    pb = probs.tile([128, 1024], BF16, name="pb", tag="pb")
                    nc.scalar.activation(pb[:], sc[:], EXP, scale=0.125)
                    pending_ctx.append((i, pb))
                    if len(pending_ctx) > 2:
                        ctx_mm(*pending_ctx.pop(0))
                for args in pending_ctx:
                    ctx_mm(*args)
                # Evacuate ctx PSUM, then compute 1/denominator via the
                # transposed layout: DMA the denom row to partition 0 of
                # dn32, block-transpose so the 1024 denoms sit 32-per-lane at
                # column stride 32, reciprocal (~0.3us), transpose back to a
                # row, cast to bf16 for the replicate matmuls.
                ctsb = small.tile([65, 1024], F32, name="ctsb", tag="ctsb")
                nc.vector.tensor_copy(ctsb[0:65, 0:512], ctA[0:65, :])
                nc.vector.tensor_copy(ctsb[0:65, 512:1024], ctB[0:65, :])
                nc.sync.dma_start(dn32[0:1, :], ctsb[64:65, :])
                nc.vector.transpose(dnT[:], dn32[:])
                nc.vector.reciprocal(
                    rT[:, 0:1024:32], dnT[:, 0:1024:32]
                )
                nc.vector.transpose(rrow[:], rT[:])
                r_bf = small.tile([1, 1024], BF16, name="rbf", tag="rbf")
                nc.vector.tensor_copy(r_bf[0:1, :], rrow[0:1, :])
                return ctsb, r_bf

            def norm_step(sq, m, ctsb, r_bf):
                rpA = pproj.tile([128, 512], F32, name="pj", tag="pj")
                rpB = pproj.tile([128, 512], F32, name="pj", tag="pj")
                nc.tensor.matmul(
                    rpA[0:64, :], ones_bf[0:1, 0:64], r_bf[0:1, 0:512],
                    start=True, stop=True,
                )
                nc.tensor.matmul(
                    rpB[0:64, :], ones_bf[0:1, 0:64], r_bf[0:1, 512:1024],
                    start=True, stop=True,
                )
                rs = small.tile([64, 1024], F32, name="rs", tag="rs")
                nc.vector.tensor_copy(rs[:, 0:512], rpA[0:64, :])
                nc.vector.tensor_copy(rs[:, 512:1024], rpB[0:64, :])
                nc.vector.tensor_mul(
                    ctxT[m][0:64, sq * 512 : (sq + 1) * 512],
                    ctsb[0:64, 0:512],
                    rs[:, 0:512],
                )
                stgB = small.tile([64, 512], BF16, name="stgB", tag="stgB")
                nc.vector.tensor_mul(
                    stgB[:], ctsb[0:64, 512:1024], rs[:, 512:1024]
                )
                nc.sync.dma_start(
                    ctxT[m][64:128, sq * 512 : (sq + 1) * 512], stgB[:]
                )

            def outproj_group(sq128, ncol, eng="v", tail=False):
                def emit():
                    po = pproj.tile([128, 512], F32, name="pj", tag="pj")
                    for m in range(NP):
                        nc.tensor.matmul(
                            po[:],
                            ctxT[m][:, sq128 * 128 : (sq128 + 1) * 128],
                            wo_t[m][:, ncol * 512 : (ncol + 1) * 512],
                            start=(m == 0),
                            stop=(m == NP - 1),
                        )
                    ob = outsb.tile([128, 512], BF16, name="ob", tag="ob")
                    if eng == "s":
                        # scalar engine is idle once the exp stream ends;
                        # alternating the tail evacuations across ACT/DVE
                        # halves the drain serialization (ACT first: the DVE
                        # is still busy with the final normalize chain)
                        nc.scalar.copy(ob[:], po[:])
                    else:
                        nc.vector.tensor_copy(ob[:], po[:])
                    dst = outp[
                        sq128 * 128 : (sq128 + 1) * 128,
                        ncol * 512 : (ncol + 1) * 512,
                    ]
                    if tail:
                        # half-stores land on two DMA rings in parallel so
                        # the last transfers don't serialize on one ring
                        nc.sync.dma_start(dst[:, 0:256], ob[:, 0:256])
                        nc.sync.dma_start(dst[:, 256:512], ob[:, 256:512])
                    else:
                        nc.sync.dma_start(dst, ob[:])

                return emit

            def outproj_groups(sq, tail=False):
                return [
                    outproj_group(
                        sq128, ncol,
                        "s" if tail and (sq128 + ncol) % 2 == 0 else "v",
                        tail,
                    )
                    for sq128 in range(sq * 4, (sq + 1) * 4)
                    for ncol in range(D // 512)
                ]

            # Two-level software pipeline: norm lags attention by one step;
            # outproj groups of a completed sq chunk are interleaved as
            # fillers inside later attention steps so they plug the PE's
            # exp-wait stalls instead of creating their own.  The V
            # projection rides the first attention step the same way (its
            # chunk i must land before that step's deferred ctx matmul i+1).
            pending = None
            fill_queue = []
            v_queue = [v_chunk(i) for i in range(SKC)]
            # q chunk nq is first read by attention step 2*nq, so its
            # projection filler releases at step nq (well after its DMA
            # lands, well before its consumer) instead of jamming the
            # PE-bound V-projection region in steps 0-1
            qsched = {
                nq: [(nq, m) for m in range(NP)] for nq in range(1, SQC)
            }
            for step in range(NP * SQC):
                sq, m = step // NP, step % NP
                fill_queue.extend(
                    q_filler(fnq, fm) for fnq, fm in qsched.get(step, [])
                )
                state = attn_step(
                    sq, m, fill_queue, pre_fillers=v_queue, filler_start=3,
                )
                if pending is not None:
                    psq, pm, pctsb, prt = pending
                    norm_step(psq, pm, pctsb, prt)
                    if pm == NP - 1:
                        fill_queue.extend(outproj_groups(psq))
                pending = (sq, m, *state)
            psq, pm, pctsb, prt = pending
            norm_step(psq, pm, pctsb, prt)
            for g in fill_queue + outproj_groups(psq, tail=True):
                g()

    nc.compile()
    return nc


def get_program():
    if "nc" not in _CACHE:
        _CACHE["nc"] = _build_program()
    return _CACHE["nc"]


def make_in_maps(q, k, v, wq, bq, wk, bk, wv, bv, wo, bo):
    q, k, v = (np.asarray(x, np.float32) for x in (q, k, v))
    wq, wk, wv, wo = (np.asarray(x, np.float32) for x in (wq, wk, wv, wo))
    bq, bk, bv = (np.asarray(x, np.float32) for x in (bq, bk, bv))
    BF = ml_dtypes.bfloat16
    qT = [np.ascontiguousarray(q[b].T).astype(BF) for b in range(B)]
    kTt = [np.ascontiguousarray(k[b].T).astype(BF) for b in range(B)]
    vTt = [np.ascontiguousarray(v[b].T).astype(BF) for b in range(B)]
    in_maps = []
    for c in range(NCORES):
        b, g = c // 4, c % 4
        sl = slice(g * GC, (g + 1) * GC)
        in_maps.append(
            {
                "qT": qT[b],
                "kT": kTt[b],
                "vT": vTt[b],
                "wqs": np.ascontiguousarray(wq[:, sl]).astype(BF),
                "wks": np.ascontiguousarray(wk[:, sl]).astype(BF),
                "wvs": np.ascontiguousarray(wv[:, sl]).astype(BF),
                "wos": np.ascontiguousarray(wo[sl, :]).astype(BF),
                "bqs": np.ascontiguousarray(bq[sl]).reshape(NP, 128, 1),
                "bks": np.ascontiguousarray(bk[sl]).reshape(NP, 128, 1),
                "bvs": np.ascontiguousarray(bv[sl]).reshape(1, GC),
            }
        )
    return in_maps


def combine_outputs(results, bo):
    out = np.zeros((B, S, D), np.float32)
    for c in range(NCORES):
        out[c // 4] += np.asarray(results[c]["outp"], np.float32)
    out += np.asarray(bo, np.float32)
    return out


def kernel(q, k, v, wq, bq, wk, bk, wv, bv, wo, bo, trace=False):
    from concourse.bass_utils import run_bass_kernel_spmd

    nc = get_program()
    in_maps = make_in_maps(q, k, v, wq, bq, wk, bk, wv, bv, wo, bo)
    res = run_bass_kernel_spmd(nc, in_maps, list(range(NCORES)), trace=trace)
    out = combine_outputs(res.results, bo)
    if trace:
        _CACHE["last_result"] = res
    return out



# revision 73
# speedup vs baseline: 1.0198x; 1.0198x over previous
"""Multi-head attention (B=2, S=2048, D=1024, H=16) on 8 Trainium2 NeuronCores.

Sharding: core c handles batch b = c//4 and head-group g = c%4 (4 heads,
a 256-wide column slice of wq/wk/wv and row slice of wo).  Each core
computes a full [S, D] bf16 partial of the output projection; the host
sums the 4 partials per batch in f32 and adds the output bias.

Per-core kernel (all layouts chosen so softmax runs over the PSUM
partition axis and every DVE/ACT op stays partition-aligned):
  - QT/KT = (x @ w)^T in [head_dim, S] layout, bf16 matmuls; bias adds
    on the scalar engine (idle during the projection phase).
  - VH = x @ wv in natural [S, head_cols] layout, bf16, with a ones
    column appended per head (yields softmax denominators for free).
  - scores^T per head-pair via row-tiled (tile_position) K=64 matmuls,
    exp on the scalar engine (scale=1/8 folded in), probs in bf16.
  - ctx^T = VH_aug^T @ probs^T accumulated over S chunks, emitted two
    iterations behind the exp stream so the PE never waits on the
    in-flight exp; row 64 of the PSUM tile is the softmax denominator.
  - normalize: denominator row -> block-transpose -> 32-lane strided
    reciprocal -> transpose back -> bf16 K=1 replicate matmul -> DVE
    multiply (a row-shaped DVE reciprocal would cost ~6.5us since DVE
    op cost scales with free-dim length, not active lanes).
  - out partial = ctx^T.T @ wo_slice, bf16 out.

Schedule: the attention spine is ACT(exp)-bound at ~1.09us per key
chunk, so everything else rides its slack: the DMA rings are loaded in
exact consumption order (k first - scores need every key - then q0, v0,
q1-3, v1, wo); only K-proj and the first q chunk run before the spine;
the other q chunks, the entire V projection, and the output-projection
groups of finished query chunks are interleaved as fillers inside the
attention steps.  Softmax normalization lags one step behind attention,
off the PE critical path.
"""

import os
import sys

import ml_dtypes
import numpy as np

if "/opt/trn_rl_repo" not in sys.path:
    sys.path.insert(0, "/opt/trn_rl_repo")

B, S, D, H = 2, 2048, 1024, 16
DH = D // H  # 64
NCORES = 8
GC = 256  # column slice per core (4 heads)
NP = 2  # head pairs per core
KC = D // 128  # 8 contraction chunks
SQC = S // 512  # 4 query chunks
SKC = S // 128  # 16 key chunks

_CACHE = {}


def _build_program():
    import concourse.bass as bass
    import concourse.tile as tile
    from concourse import bacc, mybir

    F32 = mybir.dt.float32
    F32R = mybir.dt.float32r
    BF16 = mybir.dt.bfloat16
    F8 = mybir.dt.float8e4
    DR = mybir.MatmulPerfMode.DoubleRow
    EXP = mybir.ActivationFunctionType.Exp
    PSUM = bass.MemorySpace.PSUM

    nc = bacc.Bacc()

    qT = nc.dram_tensor("qT", (D, S), BF16, kind="ExternalInput").ap()
    kT = nc.dram_tensor("kT", (D, S), BF16, kind="ExternalInput").ap()
    vT = nc.dram_tensor("vT", (D, S), BF16, kind="ExternalInput").ap()
    wqs = nc.dram_tensor("wqs", (D, GC), BF16, kind="ExternalInput").ap()
    wks = nc.dram_tensor("wks", (D, GC), BF16, kind="ExternalInput").ap()
    wvs = nc.dram_tensor("wvs", (D, GC), BF16, kind="ExternalInput").ap()
    wos = nc.dram_tensor("wos", (GC, D), BF16, kind="ExternalInput").ap()
    bqs = nc.dram_tensor("bqs", (NP, 128, 1), F32, kind="ExternalInput").ap()
    bks = nc.dram_tensor("bks", (NP, 128, 1), F32, kind="ExternalInput").ap()
    bvs = nc.dram_tensor("bvs", (1, GC), F32R, kind="ExternalInput").ap()
    outp = nc.dram_tensor("outp", (S, D), BF16, kind="ExternalOutput").ap()

    with tile.TileContext(nc) as tc:
        with (
            tc.tile_pool(name="const", bufs=1) as const,
            tc.tile_pool(name="raw", bufs=1) as rawp,
            tc.tile_pool(name="probs", bufs=7) as probs,
            tc.tile_pool(name="small", bufs=2) as small,
            tc.tile_pool(name="outsb", bufs=3) as outsb,
            tc.tile_pool(name="pproj", bufs=2, space=PSUM) as pproj,
            tc.tile_pool(name="psc", bufs=2, space=PSUM) as psc,
            tc.tile_pool(name="pctx", bufs=1, space=PSUM) as pctx,
        ):
            # ---- constants / weights ----
            # wq/wk/wv as one [128, KC, GC] bf16 tile each (single DMA)
            wq_t = const.tile([128, KC, GC], BF16, name="wqt", tag="wqt")
            wk_t = const.tile([128, KC, GC], BF16, name="wkt", tag="wkt")
            wv_t = const.tile([128, KC, GC], BF16, name="wvt", tag="wvt")
            wo_t = [const.tile([128, D], BF16, name=f"wo{m}", tag=f"wo{m}") for m in range(NP)]
            bq_t = [const.tile([128, 1], F32, name=f"bq{m}", tag=f"bq{m}") for m in range(NP)]
            bk_t = [const.tile([128, 1], F32, name=f"bk{m}", tag=f"bk{m}") for m in range(NP)]
            bv_row = const.tile([1, GC], F32R, name="bvrow", tag="bvrow")
            ones_t = const.tile([128, 128], F32R, name="ones", tag="ones")

            # PE warmup chain: keeps the HAM clock-gate at 8/8 through the
            # initial input DMA window so projections start at 2.4 GHz.
            wu = const.tile([128, 512], BF16, name="wu", tag="wu")
            nc.vector.memset(wu[:], 0.0)
            wup = psc.tile([128, 1024], F32, name="sc", tag="sc")
            for w in range(18):
                nc.tensor.matmul(
                    wup[:, 0:512], wu[:, 0:128], wu[:],
                    start=(w == 0), stop=(w == 17),
                )

            # DMA rings are FIFO: enqueue in exact consumption order.  The
            # attention spine is gated by ALL of k (scores need every key)
            # plus just the first q chunk, so k streams first; q1-3 / v1 / wo
            # are consumed later as fillers inside the attention steps.
            nc.sync.dma_start(wk_t[:], wks.rearrange("(c p) g -> p c g", p=128))
            for m in range(NP):
                nc.sync.dma_start(bk_t[m][:], bks[m])
            # bv_row is 1KB and gates the bvb broadcast matmul right before
            # the attention spine -- it must not queue behind megabytes of k/q
            nc.sync.dma_start(bv_row[:], bvs[:])
            k_t = [
                rawp.tile([128, KC, 512], BF16, name=f"k{j}", tag=f"k{j}")
                for j in range(SQC)
            ]
            kr = kT.rearrange("(c p) s -> p c s", p=128)
            for j in range(SQC):
                nc.sync.dma_start(k_t[j][:], kr[:, :, j * 512 : (j + 1) * 512])
            nc.sync.dma_start(wq_t[:], wqs.rearrange("(c p) g -> p c g", p=128))
            for m in range(NP):
                nc.sync.dma_start(bq_t[m][:], bqs[m])
            q_t = [
                rawp.tile([128, KC, 512], BF16, name=f"q{j}", tag=f"q{j}")
                for j in range(SQC)
            ]
            qr = qT.rearrange("(c p) s -> p c s", p=128)
            nc.sync.dma_start(q_t[0][:], qr[:, :, 0:512])
            nc.sync.dma_start(wv_t[:], wvs.rearrange("(c p) g -> p c g", p=128))
            v_t = [
                rawp.tile([128, KC, 1024], BF16, name=f"v{j}", tag="vv", bufs=2)
                for j in range(2)
            ]
            vr = vT.rearrange("(c p) s -> p c s", p=128)
            nc.sync.dma_start(v_t[0][:], vr[:, :, 0:1024])
            nc.sync.dma_start(v_t[1][:], vr[:, :, 1024:2048])
            for j in range(1, SQC):
                nc.sync.dma_start(q_t[j][:], qr[:, :, j * 512 : (j + 1) * 512])
            for m in range(NP):
                nc.sync.dma_start(wo_t[m][:], wos[m * 128 : (m + 1) * 128, :])
            ones_f = const.tile([128, 128], F32, name="onesf", tag="onesf")
            nc.vector.memset(ones_f[:], 1.0)
            nc.vector.tensor_copy(ones_t[:], ones_f[:])
            ones_bf = const.tile([128, 128], BF16, name="onesbf", tag="onesbf")
            nc.vector.memset(ones_bf[:], 1.0)
            # staging tiles for the transposed-reciprocal chain: the softmax
            # denominators arrive as a [1, 1024] row (partition 64 of the ctx
            # PSUM); a row-shaped DVE reciprocal costs ~6.5us (cost scales
            # with free-dim, not lanes), so we block-transpose the row across
            # 32 partitions, reciprocal 32 lanes x 32 elems, and transpose
            # back. dn32/rT are memset once so the junk lanes the transposes
            # touch stay initialized.
            dn32 = const.tile([32, 1024], F32, name="dn32", tag="dn32")
            dnT = const.tile([32, 1024], F32, name="dnT", tag="dnT")
            rT = const.tile([32, 1024], F32, name="rT", tag="rT")
            rrow = const.tile([32, 1024], F32, name="rrow", tag="rrow")
            nc.vector.memset(dn32[:], 1.0)
            nc.vector.memset(rT[:], 1.0)
            bvb = const.tile([128, GC], F32, name="bvb", tag="bvb")

            # ---- persistent activation tiles ----
            QT = [const.tile([128, S], BF16, name=f"QT{m}", tag=f"QT{m}") for m in range(NP)]
            KT = [const.tile([128, S], BF16, name=f"KT{m}", tag=f"KT{m}") for m in range(NP)]
            # VH: [S-chunk][128, 4 heads, 66] bf16; col 64 = ones, col 65 pad
            VH = [const.tile([128, 4, 66], BF16, name=f"VH{i}", tag=f"VH{i}") for i in range(SKC)]
            ctxT = [const.tile([128, S], BF16, name=f"ctxT{m}", tag=f"ctxT{m}") for m in range(NP)]

            for i in range(SKC):
                nc.vector.memset(VH[i][:, :, 64:65], 1.0)

            # ---- phase 1: projections ----
            # Bias-adds for Q/K run on the scalar engine (idle during proj)
            # so the DVE stays clear for the attention-phase chains.
            def proj_pair(raw, col0, b_t, w_t, dst, nq, m):
                ps = pproj.tile([128, 512], F32, name="pj", tag="pj")
                for k in range(KC):
                    nc.tensor.matmul(
                        ps[:],
                        w_t[:, k, m * 128 : (m + 1) * 128],
                        raw[:, k, col0 : col0 + 512],
                        start=(k == 0),
                        stop=(k == KC - 1),
                    )
                nc.scalar.activation(
                    dst[m][:, nq * 512 : (nq + 1) * 512],
                    ps[:],
                    mybir.ActivationFunctionType.Identity,
                    bias=b_t[m][:],
                )

            # K proj first (m-outer: KT[0] completes at the halfway mark),
            # then only the q chunk the first attention step reads; the other
            # three q chunks are projected as fillers inside the early
            # attention steps, overlapping their own DMA.
            for m in range(NP):
                for nq in range(SQC):
                    proj_pair(k_t[nq], 0, bk_t, wk_t, KT, nq, m)
            for m in range(NP):
                proj_pair(q_t[0], 0, bq_t, wq_t, QT, 0, m)

            def q_filler(nq, m):
                def emit():
                    proj_pair(q_t[nq], 0, bq_t, wq_t, QT, nq, m)

                return emit
            # bv broadcast to all partitions: [128, GC] = ones[1,128].T @ bv[1,GC]
            # (emitted here, not earlier: the PE's in-order queue must not
            # wait on the bv_row DMA before the Q/K projections)
            bvp = pproj.tile([128, 512], F32, name="pj", tag="pj")
            nc.tensor.matmul(
                bvp[:, :GC], ones_t[0:1, 0:128], bv_row[:],
                start=True, stop=True,
            )
            nc.vector.tensor_copy(bvb[:], bvp[:, :GC])

            def v_chunk(i):
                # one 128-key chunk of the V projection ([S, GC] layout);
                # interleaved into the first attention step's iterations
                def emit():
                    v_raw = v_t[i // 8]
                    vcol = (i % 8) * 128
                    ps = pproj.tile([128, 512], F32, name="pj", tag="pj")
                    for k in range(KC):
                        nc.tensor.matmul(
                            ps[:, :GC],
                            v_raw[:, k, vcol : vcol + 128],
                            wv_t[:, k, :],
                            start=(k == 0),
                            stop=(k == KC - 1),
                        )
                    nc.vector.tensor_add(
                        VH[i][:, :, 0:64],
                        ps[:, :GC].rearrange("p (h d) -> p h d", h=4),
                        bvb[:].rearrange("p (h d) -> p h d", h=4),
                    )

                return emit

            # ---- phase 2 + 3: software-pipelined ----
            # attn_step(sq, m) ends with a fast PSUM evacuation; the slow
            # normalize chain (1-lane reciprocal on DVE) and the dependent
            # replicate/outproj matmuls are deferred one step so the PE's
            # in-order stream never blocks on the DVE chain.
            def attn_step(
                sq, m, fillers=None, pre_fillers=None, filler_start=3,
                filler_stride=2,
            ):
                fillers = fillers if fillers is not None else []
                pre_fillers = pre_fillers if pre_fillers is not None else []
                ctA = pctx.tile([128, 512], F32, name="ctA", tag="ctA")
                ctB = pctx.tile([128, 512], F32, name="ctB", tag="ctB")

                def ctx_mm(i, pb):
                    nc.tensor.matmul(
                        ctA[0:65, :], VH[i][:, 2 * m, 0:65], pb[:, 0:512],
                        start=(i == 0), stop=(i == SKC - 1),
                    )
                    nc.tensor.matmul(
                        ctB[0:65, :], VH[i][:, 2 * m + 1, 0:65], pb[:, 512:1024],
                        start=(i == 0), stop=(i == SKC - 1),
                    )

                # ctx matmuls run two iterations behind the scores/exp: the
                # PE consumes probs(i-2) while the scalar engine computes
                # exp(i), so the PE queue never blocks on the in-flight exp,
                # and interleaved V-projection chunks get a 2-iteration
                # cushion over their DMA.
                pending_ctx = []
                for i in range(SKC):
                    if pre_fillers:
                        pre_fillers.pop(0)()
                    if (
                        i % filler_stride == filler_stride - 1
                        and i >= filler_start
                        and fillers
                    ):
                        fillers.pop(0)()
                    sc = psc.tile([128, 1024], F32, name="sc", tag="sc")
                    nc.tensor.matmul(
                        sc[:, 0:512],
                        KT[m][0:64, i * 128 : (i + 1) * 128],
                        QT[m][0:64, sq * 512 : (sq + 1) * 512],
                        start=True, stop=True,
                    )
                    nc.tensor.matmul(
                        sc[:, 512:1024],
                        KT[m][64:128, i * 128 : (i + 1) * 128],
                        QT[m][64:128, sq * 512 : (sq + 1) * 512],
                        start=True, stop=True,
                        tile_position=(64, 0),
                    )
                    pb = probs.tile([128, 1024], BF16, name="pb", tag="pb")
                    nc.scalar.activation(pb[:], sc[:], EXP, scale=0.125)
                    pending_ctx.append((i, pb))
                    if len(pending_ctx) > 2:
                        ctx_mm(*pending_ctx.pop(0))
                for args in pending_ctx:
                    ctx_mm(*args)
                # Evacuate ctx PSUM, then compute 1/denominator via the
                # transposed layout: DMA the denom row to partition 0 of
                # dn32, block-transpose so the 1024 denoms sit 32-per-lane at
                # column stride 32, reciprocal (~0.3us), transpose back to a
                # row, cast to bf16 for the replicate matmuls.
                ctsb = small.tile([65, 1024], F32, name="ctsb", tag="ctsb")
                nc.vector.tensor_copy(ctsb[0:65, 0:512], ctA[0:65, :])
                nc.vector.tensor_copy(ctsb[0:65, 512:1024], ctB[0:65, :])
                nc.sync.dma_start(dn32[0:1, :], ctsb[64:65, :])
                nc.vector.transpose(dnT[:], dn32[:])
                nc.vector.reciprocal(
                    rT[:, 0:1024:32], dnT[:, 0:1024:32]
                )
                nc.vector.transpose(rrow[:], rT[:])
                r_bf = small.tile([1, 1024], BF16, name="rbf", tag="rbf")
                nc.vector.tensor_copy(r_bf[0:1, :], rrow[0:1, :])
                return ctsb, r_bf

            def norm_step(sq, m, ctsb, r_bf):
                rpA = pproj.tile([128, 512], F32, name="pj", tag="pj")
                rpB = pproj.tile([128, 512], F32, name="pj", tag="pj")
                nc.tensor.matmul(
                    rpA[0:64, :], ones_bf[0:1, 0:64], r_bf[0:1, 0:512],
                    start=True, stop=True,
                )
                nc.tensor.matmul(
                    rpB[0:64, :], ones_bf[0:1, 0:64], r_bf[0:1, 512:1024],
                    start=True, stop=True,
                )
                rs = small.tile([64, 1024], F32, name="rs", tag="rs")
                nc.vector.tensor_copy(rs[:, 0:512], rpA[0:64, :])
                nc.vector.tensor_copy(rs[:, 512:1024], rpB[0:64, :])
                nc.vector.tensor_mul(
                    ctxT[m][0:64, sq * 512 : (sq + 1) * 512],
                    ctsb[0:64, 0:512],
                    rs[:, 0:512],
                )
                stgB = small.tile([64, 512], BF16, name="stgB", tag="stgB")
                nc.vector.tensor_mul(
                    stgB[:], ctsb[0:64, 512:1024], rs[:, 512:1024]
                )
                nc.sync.dma_start(
                    ctxT[m][64:128, sq * 512 : (sq + 1) * 512], stgB[:]
                )

            def outproj_group(sq128, ncol, eng="v"):
                def emit():
                    po = pproj.tile([128, 512], F32, name="pj", tag="pj")
                    for m in range(NP):
                        nc.tensor.matmul(
                            po[:],
                            ctxT[m][:, sq128 * 128 : (sq128 + 1) * 128],
                            wo_t[m][:, ncol * 512 : (ncol + 1) * 512],
                            start=(m == 0),
                            stop=(m == NP - 1),
                        )
                    ob = outsb.tile([128, 512], BF16, name="ob", tag="ob")
                    if eng == "s":
                        # scalar engine is idle once the exp stream ends;
                        # alternating the tail evacuations across ACT/DVE
                        # halves the drain serialization
                        nc.scalar.copy(ob[:], po[:])
                    else:
                        nc.vector.tensor_copy(ob[:], po[:])
                    nc.sync.dma_start(
                        outp[
                            sq128 * 128 : (sq128 + 1) * 128,
                            ncol * 512 : (ncol + 1) * 512,
                        ],
                        ob[:],
                    )

                return emit

            def outproj_groups(sq, tail=False):
                return [
                    outproj_group(
                        sq128, ncol, "s" if tail and (sq128 + ncol) % 2 else "v"
                    )
                    for sq128 in range(sq * 4, (sq + 1) * 4)
                    for ncol in range(D // 512)
                ]

            # Two-level software pipeline: norm lags attention by one step;
            # outproj groups of a completed sq chunk are interleaved as
            # fillers inside later attention steps so they plug the PE's
            # exp-wait stalls instead of creating their own.  The V
            # projection rides the first attention step the same way (its
            # chunk i must land before that step's deferred ctx matmul i+1).
            pending = None
            fill_queue = []
            v_queue = [v_chunk(i) for i in range(SKC)]
            # q chunk nq is first read by attention step 2*nq, so its
            # projection filler releases at step nq (well after its DMA
            # lands, well before its consumer) instead of jamming the
            # PE-bound V-projection region in steps 0-1
            qsched = {
                nq: [(nq, m) for m in range(NP)] for nq in range(1, SQC)
            }
            for step in range(NP * SQC):
                sq, m = step // NP, step % NP
                fill_queue.extend(
                    q_filler(fnq, fm) for fnq, fm in qsched.get(step, [])
                )
                state = attn_step(
                    sq, m, fill_queue, pre_fillers=v_queue, filler_start=3,
                )
                if pending is not None:
                    psq, pm, pctsb, prt = pending
                    norm_step(psq, pm, pctsb, prt)
                    if pm == NP - 1:
                        fill_queue.extend(outproj_groups(psq))
                pending = (sq, m, *state)
            psq, pm, pctsb, prt = pending
            norm_step(psq, pm, pctsb, prt)
            for g in fill_queue + outproj_groups(psq, tail=True):
                g()

    nc.compile()
    return nc


def get_program():
    if "nc" not in _CACHE:
        _CACHE["nc"] = _build_program()
    return _CACHE["nc"]


def make_in_maps(q, k, v, wq, bq, wk, bk, wv, bv, wo, bo):
    q, k, v = (np.asarray(x, np.float32) for x in (q, k, v))
    wq, wk, wv, wo = (np.asarray(x, np.float32) for x in (wq, wk, wv, wo))
    bq, bk, bv = (np.asarray(x, np.float32) for x in (bq, bk, bv))
    BF = ml_dtypes.bfloat16
    qT = [np.ascontiguousarray(q[b].T).astype(BF) for b in range(B)]
    kTt = [np.ascontiguousarray(k[b].T).astype(BF) for b in range(B)]
    vTt = [np.ascontiguousarray(v[b].T).astype(BF) for b in range(B)]
    in_maps = []
    for c in range(NCORES):
        b, g = c // 4, c % 4
        sl = slice(g * GC, (g + 1) * GC)
        in_maps.append(
            {
                "qT": qT[b],
                "kT": kTt[b],
                "vT": vTt[b],
                "wqs": np.ascontiguousarray(wq[:, sl]).astype(BF),
                "wks": np.ascontiguousarray(wk[:, sl]).astype(BF),
                "wvs": np.ascontiguousarray(wv[:, sl]).astype(BF),
                "wos": np.ascontiguousarray(wo[sl, :]).astype(BF),
                "bqs": np.ascontiguousarray(bq[sl]).reshape(NP, 128, 1),
                "bks": np.ascontiguousarray(bk[sl]).reshape(NP, 128, 1),
                "bvs": np.ascontiguousarray(bv[sl]).reshape(1, GC),
            }
        )
    return in_maps


def combine_outputs(results, bo):
    out = np.zeros((B, S, D), np.float32)
    for c in range(NCORES):
        out[c // 4] += np.asarray(results[c]["outp"], np.float32)
    out += np.asarray(bo, np.float32)
    return out


def kernel(q, k, v, wq, bq, wk, bk, wv, bv, wo, bo, trace=False):
    from concourse.bass_utils import run_bass_kernel_spmd

    nc = get_program()
    in_maps = make_in_maps(q, k, v, wq, bq, wk, bk, wv, bv, wo, bo)
    res = run_bass_kernel_spmd(nc, in_maps, list(range(NCORES)), trace=trace)
    out = combine_outputs(res.results, bo)
    if trace:
        _CACHE["last_result"] = res
    return out

